# revision 7
# baseline (speedup 1.0000x reference)
"""Trainium2 Bass kernel for the Gumbel-softmax VQ codebook model.

Computes, for B=32, N=64, D=256, C=8192:
    log_alpha = einsum('bnd,ndc->bnc', logits, head)
    z         = softmax((log_alpha + gumbel) / tau, axis=-1)
    q1        = einsum('bnc,dc->bnd', z, codebook_w)
    quantized = einsum('bnd,ndm->bnm', q1, pos_map)
    returns (quantized, log_alpha, z)

Sharding: the n_codes axis N=64 is split across 8 NeuronCores (8 positions
per core).  head / pos_map / gumbel / logits / outputs shard by n;
codebook_w is replicated.  No collectives.

Per-core dataflow (NL=8 local positions, processed as 2 "quads" of 4):
  - matmul1 per (n, dk): PSUM[32,512] chunks, lhsT = logitsT slice,
    rhs = head streamed from HBM in [128,2048] pieces.
  - PSUM chunks are copied (ScalarE) into a [128(=4n x 32b), 8192] la tile,
    DMA'd out as log_alpha, then t = la + gumbel (VectorE, in-place into the
    gumbel tile), rowwise max -> bias, exp on ScalarE with accum_out giving
    the row sum, reciprocal + rowwise scale -> z.  z DMA'd out.
  - z is transposed 128x128-blockwise on the PE and fed as stationary into
    matmul2 against the (host-pretransposed) codebook, accumulated over all
    64 C-chunks in PSUM.
  - q1 is transposed once more and contracted with pos_map per position.
"""

import os
import numpy as np

B, N, D, C = 32, 64, 256, 8192
NCORES = 8
NL = N // NCORES          # 8 local positions per core
QUADS = NL // 4           # 2 quads of 4 positions
P = 128
HEAD_PIECE = 2048         # head free-dim piece per DMA ([128, 2048] f32 = 1MB)
CJ = 512                  # matmul1 free-dim chunk (PSUM bank width in f32)

TRACE = bool(int(os.environ.get("KERNEL_TRACE", "0")))
last_results = {}


def _build_program(tau_val: float):
    import concourse.bass as bass
    import concourse.bacc as bacc
    import concourse.tile as tile
    from concourse import mybir
    from concourse.masks import make_identity
    from contextlib import ExitStack

    f32 = mybir.dt.float32
    nc = bacc.Bacc("TRN2", target_bir_lowering=False, debug=False,
                   num_devices=NCORES)

    # Per-core DRAM parameters (full tensors pre-sliced/arranged on host).
    head_d = nc.dram_tensor("head", [NL, D, C], f32, kind="ExternalInput").ap()
    gum_d = nc.dram_tensor("gumbel", [B, NL, C], f32, kind="ExternalInput").ap()
    lgT_d = nc.dram_tensor("logitsT", [P, 2 * NL * B], f32, kind="ExternalInput").ap()
    wt_d = nc.dram_tensor("wt", [P, (C // P) * D], f32, kind="ExternalInput").ap()
    pm_d = nc.dram_tensor("pm", [P, NL, 2 * D], f32, kind="ExternalInput").ap()

    la_d = nc.dram_tensor("la_out", [B, NL, C], f32, kind="ExternalOutput").ap()
    z_d = nc.dram_tensor("z_out", [B, NL, C], f32, kind="ExternalOutput").ap()
    q_d = nc.dram_tensor("q_out", [B, NL, D], f32, kind="ExternalOutput").ap()

    inv_tau = 1.0 / tau_val
    KC = C // P               # 64 chunks of the codebook contraction
    NPC = C // HEAD_PIECE     # head pieces per (n, dk)
    CJ_PER_PIECE = HEAD_PIECE // CJ

    with tile.TileContext(nc) as tc, ExitStack() as ctx:
        const_p = ctx.enter_context(tc.tile_pool(name="const", bufs=1))
        head_p = ctx.enter_context(tc.tile_pool(name="head", bufs=4))
        quad_p = ctx.enter_context(tc.tile_pool(name="quad", bufs=1))
        zq_p = ctx.enter_context(tc.tile_pool(name="zq", bufs=1))
        pm_p = ctx.enter_context(tc.tile_pool(name="pm", bufs=2))
        zt_p = ctx.enter_context(tc.tile_pool(name="zt", bufs=4))
        small_p = ctx.enter_context(tc.tile_pool(name="small", bufs=2))
        stat_p = ctx.enter_context(tc.tile_pool(name="stat", bufs=2))

        psum_mm1 = ctx.enter_context(tc.tile_pool(name="ps_mm1", bufs=4, space="PSUM"))
        psum_tr = ctx.enter_context(tc.tile_pool(name="ps_tr", bufs=2, space="PSUM"))
        psum_q1 = ctx.enter_context(tc.tile_pool(name="ps_q1", bufs=1, space="PSUM"))
        psum_q = ctx.enter_context(tc.tile_pool(name="ps_q", bufs=1, space="PSUM"))

        # --- persistent tiles -------------------------------------------------
        ident = const_p.tile([P, P], f32, tag="ident")
        make_identity(nc, ident[:])

        wt_sb = const_p.tile([P, KC * D], f32, tag="wt")
        nc.sync.dma_start(wt_sb[:], wt_d[:])

        lgT_sb = const_p.tile([P, 2, NL, B], f32, tag="lgT")
        nc.sync.dma_start(lgT_sb[:], lgT_d[:].rearrange("p (k n b) -> p k n b", k=2, n=NL))

        for q in range(QUADS):
            n0 = q * 4
            # la tile doubles as exp/z output target; g tile holds gumbel -> t.
            la_t = quad_p.tile([P, C], f32, tag="la")
            g_t = zq_p.tile([P, C], f32, tag="g")
            for qn in range(4):
                nc.sync.dma_start(g_t[qn * B:(qn + 1) * B, :],
                                  gum_d[:, n0 + qn, :])

            # ---- matmul1: log_alpha chunks --------------------------------
            for qn in range(4):
                n = n0 + qn
                for pc in range(NPC):
                    h_tiles = []
                    for dk in range(2):
                        h = head_p.tile([P, HEAD_PIECE], f32, tag="head")
                        nc.sync.dma_start(
                            h[:],
                            head_d[n, dk * P:(dk + 1) * P,
                                   pc * HEAD_PIECE:(pc + 1) * HEAD_PIECE],
                        )
                        h_tiles.append(h)
                    for j in range(CJ_PER_PIECE):
                        cj = pc * CJ_PER_PIECE + j
                        ps = psum_mm1.tile([B, CJ], f32, tag="mm1")
                        for dk in range(2):
                            nc.tensor.matmul(
                                ps[:],
                                lgT_sb[:, dk, n, :],
                                h_tiles[dk][:, j * CJ:(j + 1) * CJ],
                                start=(dk == 0), stop=(dk == 1),
                            )
                        nc.scalar.copy(la_t[qn * B:(qn + 1) * B,
                                            cj * CJ:(cj + 1) * CJ], ps[:])

            # log_alpha out
            for qn in range(4):
                nc.sync.dma_start(la_d[:, n0 + qn, :],
                                  la_t[qn * B:(qn + 1) * B, :])

            # ---- softmax ---------------------------------------------------
            # t = la + g (in-place into g tile)
            nc.vector.tensor_tensor(out=g_t[:], in0=la_t[:], in1=g_t[:],
                                    op=mybir.AluOpType.add)
            m_t = stat_p.tile([P, 1], f32, tag="m")
            nc.vector.reduce_max(m_t[:], g_t[:], axis=mybir.AxisListType.X)
            bias_t = stat_p.tile([P, 1], f32, tag="bias")
            nc.vector.tensor_scalar_mul(bias_t[:], m_t[:], -inv_tau)
            sum_t = stat_p.tile([P, 1], f32, tag="sum")
            # z(unnorm) = exp(t/tau - m/tau), row sums accumulated for free
            nc.scalar.activation(la_t[:], g_t[:],
                                 mybir.ActivationFunctionType.Exp,
                                 bias=bias_t[:], scale=inv_tau,
                                 accum_out=sum_t[:])
            inv_t = stat_p.tile([P, 1], f32, tag="inv")
            nc.vector.reciprocal(inv_t[:], sum_t[:])
            nc.vector.tensor_scalar_mul(la_t[:], la_t[:], inv_t[:])

            # z out
            for qn in range(4):
                nc.sync.dma_start(z_d[:, n0 + qn, :],
                                  la_t[qn * B:(qn + 1) * B, :])

            # ---- matmul2: q1 = z @ codebook_w.T ---------------------------
            ps_q1 = psum_q1.tile([P, D], f32, tag="q1")
            for kc in range(KC):
                ps_t = psum_tr.tile([P, P], f32, tag="tr")
                nc.tensor.transpose(ps_t[:], la_t[:, kc * P:(kc + 1) * P], ident[:])
                zT = zt_p.tile([P, P], f32, tag="zT")
                nc.scalar.copy(zT[:], ps_t[:])
                nc.tensor.matmul(ps_q1[:], zT[:], wt_sb[:, kc * D:(kc + 1) * D],
                                 start=(kc == 0), stop=(kc == KC - 1))

            q1_sb = small_p.tile([P, D], f32, tag="q1sb")
            nc.scalar.copy(q1_sb[:], ps_q1[:])
            q1T = small_p.tile([P, 2, P], f32, tag="q1T")
            for dc in range(2):
                ps_t = psum_tr.tile([P, P], f32, tag="tr")
                nc.tensor.transpose(ps_t[:], q1_sb[:, dc * P:(dc + 1) * P], ident[:])
                nc.scalar.copy(q1T[:, dc, :], ps_t[:])

            # ---- matmul3: quantized = q1 @ pos_map[n] ---------------------
            qout = small_p.tile([P, D], f32, tag="qout")
            for qn in range(4):
                n = n0 + qn
                pm_t = pm_p.tile([P, 2, D], f32, tag="pm")
                nc.sync.dma_start(
                    pm_t[:], pm_d[:, n, :].rearrange("p (k m) -> p k m", k=2))
                ps = psum_q.tile([B, D], f32, tag="q")
                for dc in range(2):
                    nc.tensor.matmul(ps[:], q1T[:, dc, qn * B:(qn + 1) * B],
                                     pm_t[:, dc, :],
                                     start=(dc == 0), stop=(dc == 1))
                nc.scalar.copy(qout[qn * B:(qn + 1) * B, :], ps[:])

            for qn in range(4):
                nc.sync.dma_start(q_d[:, n0 + qn, :],
                                  qout[qn * B:(qn + 1) * B, :])

    nc.compile()
    return nc


_prog_cache = {}


def kernel(logits, head, pos_map, codebook_w, gumbel, tau):
    from concourse.bass_utils import run_bass_kernel_spmd

    logits = np.ascontiguousarray(logits, dtype=np.float32)
    head = np.ascontiguousarray(head, dtype=np.float32)
    pos_map = np.ascontiguousarray(pos_map, dtype=np.float32)
    codebook_w = np.ascontiguousarray(codebook_w, dtype=np.float32)
    gumbel = np.ascontiguousarray(gumbel, dtype=np.float32)
    tau_val = float(np.asarray(tau).reshape(-1)[0])

    key = round(tau_val, 9)
    if key not in _prog_cache:
        _prog_cache[key] = _build_program(tau_val)
    nc = _prog_cache[key]

    # codebook_w [D, C] -> wt [128, KC*D]: wt[p, kc*D + d] = codebook_w[d, kc*128+p]
    wt_host = np.ascontiguousarray(
        codebook_w.T.reshape(C // P, P, D).transpose(1, 0, 2).reshape(P, -1))

    in_maps = []
    for i in range(NCORES):
        nsl = slice(i * NL, (i + 1) * NL)
        lg = logits[:, nsl, :]                       # [B, NL, D]
        # logitsT [p, dk, n, b] = logits[b, n, dk*128+p]
        lgT = np.ascontiguousarray(
            lg.transpose(2, 1, 0).reshape(2, P, NL, B).transpose(1, 0, 2, 3)
        ).reshape(P, -1)
        pm = pos_map[nsl]                            # [NL, D, D]
        # pm_host [p, n, dc*D + m] = pos_map[n, dc*128+p, m]
        pm_host = np.ascontiguousarray(
            pm.reshape(NL, 2, P, D).transpose(2, 0, 1, 3).reshape(P, NL, 2 * D))
        in_maps.append({
            "head": np.ascontiguousarray(head[nsl]),
            "gumbel": np.ascontiguousarray(gumbel[:, nsl, :]),
            "logitsT": lgT,
            "wt": wt_host,
            "pm": pm_host,
        })

    trace_kwargs = {}
    if TRACE:
        trace_dir = os.environ.get("KERNEL_TRACE_DIR")
        if trace_dir:
            os.makedirs(trace_dir, exist_ok=True)
            trace_kwargs["tmpdir"] = trace_dir
    res = run_bass_kernel_spmd(nc, in_maps, list(range(NCORES)), trace=TRACE,
                               **trace_kwargs)
    last_results["exec_time_ns"] = res.exec_time_ns
    last_results["profile_json"] = res.profile_json
    last_results["instructions_and_trace"] = res.instructions_and_trace

    quantized = np.concatenate([res.results[i]["q_out"] for i in range(NCORES)], axis=1)
    log_alpha = np.concatenate([res.results[i]["la_out"] for i in range(NCORES)], axis=1)
    z = np.concatenate([res.results[i]["z_out"] for i in range(NCORES)], axis=1)
    return (quantized, log_alpha, z)


# revision 9
# speedup vs baseline: 1.1010x; 1.1010x over previous
"""Trainium2 Bass kernel for the Gumbel-softmax VQ codebook model.

Computes, for B=32, N=64, D=256, C=8192:
    log_alpha = einsum('bnd,ndc->bnc', logits, head)
    z         = softmax((log_alpha + gumbel) / tau, axis=-1)
    q1        = einsum('bnc,dc->bnd', z, codebook_w)
    quantized = einsum('bnd,ndm->bnm', q1, pos_map)
    returns (quantized, log_alpha, z)

Sharding: the n_codes axis N=64 is split across 8 NeuronCores (8 positions
per core).  head / pos_map / gumbel / logits / outputs shard by n;
codebook_w is replicated.  No collectives.

fp32 matmuls on the PE run as LOW_HIGH pairs at 2 cycles/column (4x the
fp16 cost), so the two big contractions run in split precision instead:
x is decomposed on the host into fp16 hi = fp16(x) and lo = fp16(x - hi),
and the product keeps the hi*hi + lo*hi + hi*lo terms (the dropped lo*lo
term is ~2^-22 relative).  For matmul1 the stationary operand stacks
[lg_hi | lg_lo] along M so one h_hi stream produces both hi*hi and lo*hi;
the h_lo stream adds hi*lo.  PSUM accumulates strips [0:32] and [32:64],
merged by a single VectorE add during eviction.
"""

import os
import numpy as np

B, N, D, C = 32, 64, 256, 8192
NCORES = 8
NL = N // NCORES          # 8 local positions per core
QUADS = NL // 4           # 2 quads of 4 positions
P = 128
HEAD_PIECE = 4096         # head free-dim piece per DMA ([128, 4096] fp16 = 1MB)
CJ = 512                  # matmul1 free-dim chunk (PSUM bank width in f32)

TRACE = bool(int(os.environ.get("KERNEL_TRACE", "0")))
last_results = {}


def _build_program(tau_val: float):
    import concourse.bass as bass
    import concourse.bacc as bacc
    import concourse.tile as tile
    from concourse import mybir
    from concourse.masks import make_identity
    from contextlib import ExitStack

    f32 = mybir.dt.float32
    f16 = mybir.dt.float16
    nc = bacc.Bacc("TRN2", target_bir_lowering=False, debug=False,
                   num_devices=NCORES)

    head_hi_d = nc.dram_tensor("head_hi", [NL, D, C], f16, kind="ExternalInput").ap()
    head_lo_d = nc.dram_tensor("head_lo", [NL, D, C], f16, kind="ExternalInput").ap()
    gum_d = nc.dram_tensor("gumbel", [B, NL, C], f32, kind="ExternalInput").ap()
    lgT_d = nc.dram_tensor("logitsT", [P, 2 * NL * 2 * B], f16, kind="ExternalInput").ap()
    wt_hi_d = nc.dram_tensor("wt_hi", [P, (C // P) * D], f16, kind="ExternalInput").ap()
    wt_lo_d = nc.dram_tensor("wt_lo", [P, (C // P) * D], f16, kind="ExternalInput").ap()
    pm_d = nc.dram_tensor("pm", [P, NL, 2 * D], f32, kind="ExternalInput").ap()

    la_d = nc.dram_tensor("la_out", [B, NL, C], f32, kind="ExternalOutput").ap()
    z_d = nc.dram_tensor("z_out", [B, NL, C], f32, kind="ExternalOutput").ap()
    q_d = nc.dram_tensor("q_out", [B, NL, D], f32, kind="ExternalOutput").ap()

    inv_tau = 1.0 / tau_val
    KC = C // P               # 64 chunks of the codebook contraction
    NPC = C // HEAD_PIECE     # head pieces per (n, dk)
    CJ_PER_PIECE = HEAD_PIECE // CJ

    with tile.TileContext(nc) as tc, ExitStack() as ctx:
        const_p = ctx.enter_context(tc.tile_pool(name="const", bufs=1))
        head_p = ctx.enter_context(tc.tile_pool(name="head", bufs=4))
        quad_p = ctx.enter_context(tc.tile_pool(name="quad", bufs=1))
        zq_p = ctx.enter_context(tc.tile_pool(name="zq", bufs=1))
        pm_p = ctx.enter_context(tc.tile_pool(name="pm", bufs=2))
        zt_p = ctx.enter_context(tc.tile_pool(name="zt", bufs=4))
        small_p = ctx.enter_context(tc.tile_pool(name="small", bufs=2))
        stat_p = ctx.enter_context(tc.tile_pool(name="stat", bufs=2))

        psum_mm1 = ctx.enter_context(tc.tile_pool(name="ps_mm1", bufs=4, space="PSUM"))
        psum_tr = ctx.enter_context(tc.tile_pool(name="ps_tr", bufs=2, space="PSUM"))
        psum_q1 = ctx.enter_context(tc.tile_pool(name="ps_q1", bufs=1, space="PSUM"))
        psum_q = ctx.enter_context(tc.tile_pool(name="ps_q", bufs=1, space="PSUM"))

        # --- persistent tiles -------------------------------------------------
        ident = const_p.tile([P, P], f32, tag="ident")
        make_identity(nc, ident[:])

        wt_hi = const_p.tile([P, KC * D], f16, tag="wt_hi")
        nc.sync.dma_start(wt_hi[:], wt_hi_d[:])
        wt_lo = const_p.tile([P, KC * D], f16, tag="wt_lo")
        nc.sync.dma_start(wt_lo[:], wt_lo_d[:])

        # [p, dk, n, 0:32]=lg_hi, [p, dk, n, 32:64]=lg_lo
        lgT_sb = const_p.tile([P, 2, NL, 2 * B], f16, tag="lgT")
        nc.sync.dma_start(
            lgT_sb[:], lgT_d[:].rearrange("p (k n b) -> p k n b", k=2, n=NL))

        for q in range(QUADS):
            n0 = q * 4
            la_t = quad_p.tile([P, C], f32, tag="la")
            g_t = zq_p.tile([P, C], f32, tag="g")
            for qn in range(4):
                nc.sync.dma_start(g_t[qn * B:(qn + 1) * B, :],
                                  gum_d[:, n0 + qn, :])

            # ---- matmul1: log_alpha chunks --------------------------------
            for qn in range(4):
                n = n0 + qn
                for pc in range(NPC):
                    hh, hl = [], []
                    for dk in range(2):
                        th = head_p.tile([P, HEAD_PIECE], f16, tag="h_hi")
                        nc.sync.dma_start(
                            th[:],
                            head_hi_d[n, dk * P:(dk + 1) * P,
                                      pc * HEAD_PIECE:(pc + 1) * HEAD_PIECE])
                        hh.append(th)
                        tl = head_p.tile([P, HEAD_PIECE], f16, tag="h_lo")
                        nc.sync.dma_start(
                            tl[:],
                            head_lo_d[n, dk * P:(dk + 1) * P,
                                      pc * HEAD_PIECE:(pc + 1) * HEAD_PIECE])
                        hl.append(tl)
                    for j in range(CJ_PER_PIECE):
                        cj = pc * CJ_PER_PIECE + j
                        ps = psum_mm1.tile([2 * B, CJ], f32, tag="mm1")
                        for dk in range(2):
                            # [lg_hi | lg_lo] @ h_hi -> strips [0:32],[32:64]
                            nc.tensor.matmul(
                                ps[:], lgT_sb[:, dk, n, :],
                                hh[dk][:, j * CJ:(j + 1) * CJ],
                                start=(dk == 0), stop=False,
                                skip_group_check=True)
                        for dk in range(2):
                            # lg_hi @ h_lo -> strip [0:32]
                            nc.tensor.matmul(
                                ps[0:B, :], lgT_sb[:, dk, n, 0:B],
                                hl[dk][:, j * CJ:(j + 1) * CJ],
                                start=False, stop=(dk == 1),
                                skip_group_check=True)
                        # merge strips: la = ps[0:32] + ps[32:64]
                        # (two PSUM operands in one op are illegal: NCC_IBVF027)
                        la_sl = la_t[qn * B:(qn + 1) * B, cj * CJ:(cj + 1) * CJ]
                        nc.scalar.copy(la_sl, ps[B:2 * B, :])
                        nc.vector.tensor_tensor(out=la_sl, in0=ps[0:B, :],
                                                in1=la_sl,
                                                op=mybir.AluOpType.add)

            # log_alpha out
            for qn in range(4):
                nc.sync.dma_start(la_d[:, n0 + qn, :],
                                  la_t[qn * B:(qn + 1) * B, :])

            # ---- softmax ---------------------------------------------------
            nc.vector.tensor_tensor(out=g_t[:], in0=la_t[:], in1=g_t[:],
                                    op=mybir.AluOpType.add)
            m_t = stat_p.tile([P, 1], f32, tag="m")
            nc.vector.reduce_max(m_t[:], g_t[:], axis=mybir.AxisListType.X)
            bias_t = stat_p.tile([P, 1], f32, tag="bias")
            nc.vector.tensor_scalar_mul(bias_t[:], m_t[:], -inv_tau)
            sum_t = stat_p.tile([P, 1], f32, tag="sum")
            nc.scalar.activation(la_t[:], g_t[:],
                                 mybir.ActivationFunctionType.Exp,
                                 bias=bias_t[:], scale=inv_tau,
                                 accum_out=sum_t[:])
            inv_t = stat_p.tile([P, 1], f32, tag="inv")
            nc.vector.reciprocal(inv_t[:], sum_t[:])
            nc.vector.tensor_scalar_mul(la_t[:], la_t[:], inv_t[:])

            # z out
            for qn in range(4):
                nc.sync.dma_start(z_d[:, n0 + qn, :],
                                  la_t[qn * B:(qn + 1) * B, :])

            # ---- matmul2: q1 = z @ codebook_w.T ---------------------------
            ps_q1 = psum_q1.tile([P, D], f32, tag="q1")
            for kc in range(KC):
                ps_t = psum_tr.tile([P, P], f32, tag="tr")
                nc.tensor.transpose(ps_t[:], la_t[:, kc * P:(kc + 1) * P], ident[:])
                zT_hi = zt_p.tile([P, P], f16, tag="zT_hi")
                nc.scalar.copy(zT_hi[:], ps_t[:])
                zT_lo = zt_p.tile([P, P], f16, tag="zT_lo")
                nc.vector.tensor_tensor(out=zT_lo[:], in0=ps_t[:], in1=zT_hi[:],
                                        op=mybir.AluOpType.subtract)
                nc.tensor.matmul(ps_q1[:], zT_hi[:], wt_hi[:, kc * D:(kc + 1) * D],
                                 start=(kc == 0), stop=False,
                                 skip_group_check=True)
                nc.tensor.matmul(ps_q1[:], zT_lo[:], wt_hi[:, kc * D:(kc + 1) * D],
                                 start=False, stop=False,
                                 skip_group_check=True)
                nc.tensor.matmul(ps_q1[:], zT_hi[:], wt_lo[:, kc * D:(kc + 1) * D],
                                 start=False, stop=(kc == KC - 1),
                                 skip_group_check=True)

            q1_sb = small_p.tile([P, D], f32, tag="q1sb")
            nc.scalar.copy(q1_sb[:], ps_q1[:])
            q1T = small_p.tile([P, 2, P], f32, tag="q1T")
            for dc in range(2):
                ps_t = psum_tr.tile([P, P], f32, tag="tr")
                nc.tensor.transpose(ps_t[:], q1_sb[:, dc * P:(dc + 1) * P], ident[:])
                nc.scalar.copy(q1T[:, dc, :], ps_t[:])

            # ---- matmul3: quantized = q1 @ pos_map[n] ---------------------
            qout = small_p.tile([P, D], f32, tag="qout")
            for qn in range(4):
                n = n0 + qn
                pm_t = pm_p.tile([P, 2, D], f32, tag="pm")
                nc.sync.dma_start(
                    pm_t[:], pm_d[:, n, :].rearrange("p (k m) -> p k m", k=2))
                ps = psum_q.tile([B, D], f32, tag="q")
                for dc in range(2):
                    nc.tensor.matmul(ps[:], q1T[:, dc, qn * B:(qn + 1) * B],
                                     pm_t[:, dc, :],
                                     start=(dc == 0), stop=(dc == 1))
                nc.scalar.copy(qout[qn * B:(qn + 1) * B, :], ps[:])

            for qn in range(4):
                nc.sync.dma_start(q_d[:, n0 + qn, :],
                                  qout[qn * B:(qn + 1) * B, :])

    nc.compile()
    return nc


def _split16(x):
    hi = x.astype(np.float16)
    lo = (x - hi.astype(np.float32)).astype(np.float16)
    return hi, lo


def _host_arrange(logits, head, pos_map, codebook_w, gumbel):
    """Build the 8 per-core input maps (host-side layout shuffles)."""
    wt_hi_f, wt_lo_f = _split16(
        np.ascontiguousarray(codebook_w.T.reshape(C // P, P, D)
                             .transpose(1, 0, 2).reshape(P, -1)))
    head_hi, head_lo = _split16(head)
    in_maps = []
    for i in range(NCORES):
        nsl = slice(i * NL, (i + 1) * NL)
        lg = logits[:, nsl, :]                       # [B, NL, D]
        # lgT [p, dk, n, b] = logits[b, n, dk*128+p]
        lgT = np.ascontiguousarray(
            lg.transpose(2, 1, 0).reshape(2, P, NL, B).transpose(1, 0, 2, 3))
        lgT_hi, lgT_lo = _split16(lgT)
        lgT_stack = np.concatenate([lgT_hi, lgT_lo], axis=3)  # [P,2,NL,64]
        pm = pos_map[nsl]                            # [NL, D, D]
        pm_host = np.ascontiguousarray(
            pm.reshape(NL, 2, P, D).transpose(2, 0, 1, 3).reshape(P, NL, 2 * D))
        in_maps.append({
            "head_hi": np.ascontiguousarray(head_hi[nsl]),
            "head_lo": np.ascontiguousarray(head_lo[nsl]),
            "gumbel": np.ascontiguousarray(gumbel[:, nsl, :]),
            "logitsT": np.ascontiguousarray(lgT_stack).reshape(P, -1),
            "wt_hi": wt_hi_f,
            "wt_lo": wt_lo_f,
            "pm": pm_host,
        })
    return in_maps


_prog_cache = {}


def kernel(logits, head, pos_map, codebook_w, gumbel, tau):
    from concourse.bass_utils import run_bass_kernel_spmd

    logits = np.ascontiguousarray(logits, dtype=np.float32)
    head = np.ascontiguousarray(head, dtype=np.float32)
    pos_map = np.ascontiguousarray(pos_map, dtype=np.float32)
    codebook_w = np.ascontiguousarray(codebook_w, dtype=np.float32)
    gumbel = np.ascontiguousarray(gumbel, dtype=np.float32)
    tau_val = float(np.asarray(tau).reshape(-1)[0])

    key = round(tau_val, 9)
    if key not in _prog_cache:
        _prog_cache[key] = _build_program(tau_val)
    nc = _prog_cache[key]

    in_maps = _host_arrange(logits, head, pos_map, codebook_w, gumbel)

    trace_kwargs = {}
    if TRACE:
        trace_dir = os.environ.get("KERNEL_TRACE_DIR")
        if trace_dir:
            os.makedirs(trace_dir, exist_ok=True)
            trace_kwargs["tmpdir"] = trace_dir
    res = run_bass_kernel_spmd(nc, in_maps, list(range(NCORES)), trace=TRACE,
                               **trace_kwargs)
    last_results["exec_time_ns"] = res.exec_time_ns
    last_results["profile_json"] = res.profile_json
    last_results["instructions_and_trace"] = res.instructions_and_trace

    quantized = np.concatenate([res.results[i]["q_out"] for i in range(NCORES)], axis=1)
    log_alpha = np.concatenate([res.results[i]["la_out"] for i in range(NCORES)], axis=1)
    z = np.concatenate([res.results[i]["z_out"] for i in range(NCORES)], axis=1)
    return (quantized, log_alpha, z)
